# revision 1
# baseline (speedup 1.0000x reference)
"""Trainium2 Bass kernel for nn_CustomConv2d: 3x3 conv, stride 1, pad 1.

x: [32, 128, 56, 56] f32, kernel: [256, 128, 3, 3] f32, bias: [256] f32
-> out: [32, 256, 56, 56] f32

Strategy: data-parallel over batch (4 images per core on 8 cores).
Per core the conv is 9 accumulating matmuls per output tile:
  psum[co_blk, pix] += W[kh,kw][ci, co_blk].T @ xpad[ci, shifted pix]
with C_in = 128 = the PE contraction dim, C_out split into 2 blocks of
128 partitions, and pixels tiled 8 output rows (448) at a time into one
PSUM bank. x is zero-padded on host to [58, 58] and loaded per 10-row
chunk (8 output rows + 2 halo) so compute overlaps the loads tightly.

Matmuls run in float32r (TF32-like, 11-bit mantissa; ~1e-4 rel err,
4x the fp32 PE rate). The PE rounds f32r inputs itself, so raw fp32
bits are DMA'd unchanged into f32r SBUF tiles.
"""

import sys

import numpy as np

try:
    import concourse  # noqa: F401  (provided on PYTHONPATH via axon site)
except ImportError:
    sys.path.insert(0, "/opt/trn_rl_repo")

import concourse.bass as bass
import concourse.mybir as mybir
import concourse.tile as tile
from concourse import bacc
from concourse.bass_utils import run_bass_kernel_spmd

B, C_IN, C_OUT, KS, H, W = 32, 128, 256, 3, 56, 56
N_CORES = 8
B_LOC = B // N_CORES
HP, WP = H + 2, W + 2
ROWS_PER_TILE = 8
N_TILE = ROWS_PER_TILE * W  # 448 <= 512 (one fp32 PSUM bank)
N_TILES = H // ROWS_PER_TILE  # 7
CHUNK_ROWS = ROWS_PER_TILE + 2  # padded rows per x chunk (with halo)
CO_BLOCKS = C_OUT // 128

MODE = "f32r"  # "f32" | "f32r" | "bf16"
GROUP2 = False  # pair two PSUM tiles per weight load (k-outer in groups)
STORE_ENG = "pool"  # "pool" (gpsimd/SWDGE) | "act" (scalar/HWDGE)

_NC_CACHE: dict = {}


def _build_cached(mode: str, repeats: int = 1) -> bass.Bass:
    key = (mode, repeats)
    if key not in _NC_CACHE:
        _NC_CACHE[key] = _build(mode, repeats)
    return _NC_CACHE[key]


def _build(mode: str, repeats: int = 1) -> bass.Bass:
    f32 = mybir.dt.float32
    if mode == "bf16":
        sb_dt = mybir.dt.bfloat16
    elif mode in ("f32r", "mixw"):
        sb_dt = mybir.dt.float32r
    else:
        sb_dt = f32
    # "mixw": bf16 weights (half the PE weight-load stream) + f32r activations
    w_dt = mybir.dt.bfloat16 if mode == "mixw" else sb_dt

    nc = bacc.Bacc("TRN2", target_bir_lowering=False, debug=False)
    xp_d = nc.dram_tensor("xp", [B_LOC, C_IN, HP, WP], sb_dt, kind="ExternalInput").ap()
    w_d = nc.dram_tensor("w", [C_IN, 9 * C_OUT], w_dt, kind="ExternalInput").ap()
    b_d = nc.dram_tensor("bias", [128, CO_BLOCKS], f32, kind="ExternalInput").ap()
    out_d = nc.dram_tensor("out", [B_LOC, C_OUT, H, W], f32, kind="ExternalOutput").ap()
    out_flat = out_d.rearrange("b c h w -> b c (h w)")
    xp_rows = xp_d.rearrange("b c h w -> b c (h w)")

    # x chunks per image: A covers output rows 0..15 (padded rows 0..17),
    # B rows 16..31 (padded 16..33), C rows 32..55 (padded 32..57).
    CHUNKS = [(0, 18, (0, 1)), (16, 18, (2, 3)), (32, 26, (4, 5, 6))]
    t2chunk = {}
    for ci, (r0, nr, ts_) in enumerate(CHUNKS):
        for t in ts_:
            t2chunk[t] = (ci, r0)

    with tile.TileContext(nc) as tc:
        with (
            tc.tile_pool(name="const", bufs=1) as const,
            tc.tile_pool(name="xpool", bufs=6) as xpool,
            tc.tile_pool(name="opool", bufs=4) as opool,
            tc.tile_pool(name="psum", bufs=8, space="PSUM") as psum,
        ):
            import contextlib

            loop_cm = (
                tc.For_i(0, repeats, 1, hint_engines=(mybir.EngineType.PE,))
                if repeats > 1
                else contextlib.nullcontext()
            )
            with loop_cm:
                # per-co-block weight tiles: first matmul gates on a 0.6MB DMA.
                # Emission order interleaves image-0's first chunk right after
                # w0 so the PE can start ~2us in; bias is only needed by the
                # first eviction so it loads last.
                wco = [
                    const.tile([C_IN, 9 * 128], w_dt, tag=f"w{co}", name=f"w{co}")
                    for co in range(CO_BLOCKS)
                ]
                bt = const.tile([128, CO_BLOCKS], f32)
                nc.sync.dma_start(wco[0][:], w_d[:, : 9 * 128])
                xc0 = []
                for i, (r0, nr, _ts) in enumerate(CHUNKS):
                    xt = xpool.tile([C_IN, 26, WP], sb_dt, tag="xt", name="xt")
                    nc.sync.dma_start(
                        xt[:, :nr, :], xp_rows[0, :, r0 * WP : (r0 + nr) * WP]
                    )
                    xc0.append(xt)
                    if i == 0:
                        nc.sync.dma_start(wco[1][:], w_d[:, 9 * 128 :])
                nc.sync.dma_start(bt[:], b_d[:])

                for b in range(B_LOC):
                    if b == 0:
                        xc = xc0
                    else:
                        xc = []
                        for r0, nr, _ts in CHUNKS:
                            xt = xpool.tile([C_IN, 26, WP], sb_dt, tag="xt", name="xt")
                            nc.sync.dma_start(
                                xt[:, :nr, :], xp_rows[b, :, r0 * WP : (r0 + nr) * WP]
                            )
                            xc.append(xt)
                    t_groups = (
                        [(0, 1), (2, 3), (4, 5), (6,)]
                        if GROUP2
                        else [(t,) for t in range(N_TILES)]
                    )
                    for tg in t_groups:
                        for co in range(CO_BLOCKS):
                            pts = [
                                psum.tile([128, N_TILE], f32, tag="pt", name="pt")
                                for _ in tg
                            ]
                            for k in range(9):
                                kh, kw = divmod(k, KS)
                                for j, t in enumerate(tg):
                                    h0 = ROWS_PER_TILE * t
                                    ci, r0 = t2chunk[t]
                                    lr = h0 - r0
                                    rhs = xc[ci][:, lr + kh : lr + kh + ROWS_PER_TILE, kw : kw + W]
                                    nc.tensor.matmul(
                                        pts[j][:],
                                        wco[co][:, k * 128 : (k + 1) * 128],
                                        rhs,
                                        start=(k == 0),
                                        stop=(k == 8),
                                    )
                            for j, t in enumerate(tg):
                                h0 = ROWS_PER_TILE * t
                                ot = opool.tile([128, N_TILE], f32)
                                nc.vector.tensor_scalar_add(ot[:], pts[j][:], bt[:, co : co + 1])
                                store_eng = nc.gpsimd if STORE_ENG == "pool" else nc.scalar
                                store_eng.dma_start(
                                    out_flat[b, co * 128 : (co + 1) * 128, h0 * W : h0 * W + N_TILE],
                                    ot[:],
                                )
    nc.compile()
    return nc


def _host_prep(x, kernel, bias, mode: str):
    np_dt = np.float32
    w_np_dt = np.float32
    if mode in ("bf16", "mixw"):
        import ml_dtypes

        w_np_dt = ml_dtypes.bfloat16
        if mode == "bf16":
            np_dt = ml_dtypes.bfloat16

    xp = np.zeros((B, C_IN, HP, WP), dtype=np_dt)
    xp[:, :, 1 : 1 + H, 1 : 1 + W] = x
    # w[co, ci, kh, kw] -> w_t[ci, co_blk*9*128 + (kh*3+kw)*128 + co_in]
    w5 = kernel.reshape(CO_BLOCKS, 128, C_IN, KS, KS)
    w_t = np.ascontiguousarray(
        w5.transpose(2, 0, 3, 4, 1).reshape(C_IN, 9 * C_OUT).astype(w_np_dt)
    )
    b_t = np.ascontiguousarray(bias.astype(np.float32).reshape(CO_BLOCKS, 128).T)
    return xp, w_t, b_t


def kernel(x, kernel, bias):  # noqa: A002 - names fixed by harness contract
    x = np.asarray(x, dtype=np.float32)
    kernel = np.asarray(kernel, dtype=np.float32)
    bias = np.asarray(bias, dtype=np.float32)

    nc = _build_cached(MODE)
    xp, w_t, b_t = _host_prep(x, kernel, bias, MODE)
    in_maps = [
        {"xp": xp[c * B_LOC : (c + 1) * B_LOC], "w": w_t, "bias": b_t}
        for c in range(N_CORES)
    ]
    res = run_bass_kernel_spmd(nc, in_maps, core_ids=list(range(N_CORES)))
    out = np.concatenate([r["out"] for r in res.results], axis=0)
    return out



# revision 6
# speedup vs baseline: 1.1997x; 1.1997x over previous
"""Trainium2 Bass kernel for nn_CustomConv2d: 3x3 conv, stride 1, pad 1.

x: [32, 128, 56, 56] f32, kernel: [256, 128, 3, 3] f32, bias: [256] f32
-> out: [32, 256, 56, 56] f32

Strategy: data-parallel over batch (4 images per core on 8 cores).
Per core the conv is 9 accumulating matmuls per output tile:
  psum[co_blk, pix] += W[kh,kw][ci, co_blk].T @ xpad[ci, shifted pix]
with C_in = 128 = the PE contraction dim, C_out split into 2 blocks of
128 partitions, and pixels tiled 8 output rows (448) at a time into one
PSUM bank. x is zero-padded on host to [58, 58] and loaded per 10-row
chunk (8 output rows + 2 halo) so compute overlaps the loads tightly.

Matmuls run in float32r (TF32-like, 11-bit mantissa; ~1e-4 rel err,
4x the fp32 PE rate). The PE rounds f32r inputs itself, so raw fp32
bits are DMA'd unchanged into f32r SBUF tiles.
"""

import sys

import numpy as np

try:
    import concourse  # noqa: F401  (provided on PYTHONPATH via axon site)
except ImportError:
    sys.path.insert(0, "/opt/trn_rl_repo")

import concourse.bass as bass
import concourse.mybir as mybir
import concourse.tile as tile
from concourse import bacc
from concourse.bass_utils import run_bass_kernel_spmd

B, C_IN, C_OUT, KS, H, W = 32, 128, 256, 3, 56, 56
N_CORES = 8
B_LOC = B // N_CORES
HP, WP = H + 2, W + 2
ROWS_PER_TILE = 8
N_TILE = ROWS_PER_TILE * W  # 448 <= 512 (one fp32 PSUM bank)
N_TILES = H // ROWS_PER_TILE  # 7
CHUNK_ROWS = ROWS_PER_TILE + 2  # padded rows per x chunk (with halo)
CO_BLOCKS = C_OUT // 128

MODE = "f32r"  # "f32" | "f32r" | "bf16" | "f32rc" | "bf16c"
GROUP2 = False  # pair two PSUM tiles per weight load (k-outer in groups)
GROUP = 1  # contiguous-window path: tiles per weight load (1, 2)
STORE_ENG = "pool"  # "pool" (gpsimd/SWDGE) | "act" (scalar/HWDGE)

_NC_CACHE: dict = {}


def _build_cached(mode: str, repeats: int = 1) -> bass.Bass:
    key = (mode, repeats, GROUP)
    if key not in _NC_CACHE:
        _NC_CACHE[key] = _build(mode, repeats)
    return _NC_CACHE[key]


N_WIN = 8 * WP  # 464: 8 padded rows incl. 2 garbage cols per row


def _build_contig(mode: str, repeats: int = 1) -> bass.Bass:
    """Contiguous-window path: each tap's rhs is a flat 464-elem slice of the
    row-major padded image; outputs land on a 58-col grid in PSUM and the
    eviction skips the 2 garbage columns per row."""
    f32 = mybir.dt.float32
    sb_dt = mybir.dt.bfloat16 if mode == "bf16c" else mybir.dt.float32r
    w_dt = sb_dt

    nc = bacc.Bacc("TRN2", target_bir_lowering=False, debug=False)
    xp_d = nc.dram_tensor("xp", [B_LOC, C_IN, HP * WP], sb_dt, kind="ExternalInput").ap()
    w_d = nc.dram_tensor("w", [C_IN, 9 * C_OUT], w_dt, kind="ExternalInput").ap()
    b_d = nc.dram_tensor("bias", [128, CO_BLOCKS], f32, kind="ExternalInput").ap()
    out_d = nc.dram_tensor("out", [B_LOC, C_OUT, H, W], f32, kind="ExternalOutput").ap()
    out_flat = out_d.rearrange("b c h w -> b c (h w)")

    # x chunks per image: (padded start row, n padded rows, output tiles)
    CHUNKS = [(0, 18, (0, 1)), (16, 18, (2, 3)), (32, 26, (4, 5, 6))]
    t2chunk = {}
    for ci, (r0, nr, ts_) in enumerate(CHUNKS):
        for t in ts_:
            t2chunk[t] = (ci, r0)
    if GROUP == 2:
        t_groups = [(0, 1), (2, 3), (4, 5), (6,)]
    else:
        t_groups = [(t,) for t in range(N_TILES)]

    with tile.TileContext(nc) as tc:
        with (
            tc.tile_pool(name="const", bufs=1) as const,
            tc.tile_pool(name="xpool", bufs=6) as xpool,
            tc.tile_pool(name="opool", bufs=4) as opool,
            tc.tile_pool(name="psum", bufs=8, space="PSUM") as psum,
        ):
            import contextlib

            # weights + bias are loop-invariant: load once, before the loop
            wco = [
                const.tile([C_IN, 9 * 128], w_dt, tag=f"w{co}", name=f"w{co}")
                for co in range(CO_BLOCKS)
            ]
            bt = const.tile([128, CO_BLOCKS], f32)
            for co in range(CO_BLOCKS):
                nc.sync.dma_start(wco[co][:], w_d[:, co * 9 * 128 : (co + 1) * 9 * 128])
            nc.sync.dma_start(bt[:], b_d[:])

            loop_cm = (
                tc.For_i(0, repeats, 1, hint_engines=(mybir.EngineType.PE,))
                if repeats > 1
                else contextlib.nullcontext()
            )
            with loop_cm:
                for b in range(B_LOC):
                    xc = []
                    for r0, nr, _ts in CHUNKS:
                        # +2 slop: last tap window of the last tile in a chunk
                        # overruns the loaded rows by 2 elems (garbage cols only)
                        xt = xpool.tile([C_IN, 26 * WP + 2], sb_dt, tag="xt", name="xt")
                        nc.sync.dma_start(
                            xt[:, : nr * WP], xp_d[b, :, r0 * WP : (r0 + nr) * WP]
                        )
                        nc.vector.memset(xt[:, nr * WP : nr * WP + 2], 0.0)
                        xc.append(xt)
                    for tg in t_groups:
                        for co in range(CO_BLOCKS):
                            pts = [
                                psum.tile([128, N_WIN], f32, tag="pt", name="pt")
                                for _ in tg
                            ]
                            for k in range(9):
                                kh, kw = divmod(k, KS)
                                for j, t in enumerate(tg):
                                    ci, r0 = t2chunk[t]
                                    lr = ROWS_PER_TILE * t - r0
                                    base = (lr + kh) * WP + kw
                                    nc.tensor.matmul(
                                        pts[j][:],
                                        wco[co][:, k * 128 : (k + 1) * 128],
                                        xc[ci][:, base : base + N_WIN],
                                        start=(k == 0),
                                        stop=(k == 8),
                                    )
                            for j, t in enumerate(tg):
                                h0 = ROWS_PER_TILE * t
                                ot = opool.tile([128, N_TILE], f32)
                                pv = pts[j][:].rearrange(
                                    "p (h w) -> p h w", h=ROWS_PER_TILE
                                )[:, :, :W]
                                nc.vector.tensor_scalar_add(ot[:], pv, bt[:, co : co + 1])
                                store_eng = nc.gpsimd if STORE_ENG == "pool" else nc.scalar
                                store_eng.dma_start(
                                    out_flat[b, co * 128 : (co + 1) * 128, h0 * W : h0 * W + N_TILE],
                                    ot[:],
                                )
    nc.compile()
    return nc


def _build_packed(mode: str, repeats: int = 1) -> bass.Bass:
    """Packed-copies path: three kw-shifted dense copies of the padded image
    (row pitch exactly 56) so every tap's rhs is a contiguous, 16B-aligned,
    448-element window. Zero wasted stream rows; eviction reads PSUM flat."""
    f32 = mybir.dt.float32
    sb_dt = mybir.dt.bfloat16 if mode == "bf16p" else mybir.dt.float32r
    w_dt = sb_dt
    PITCH = W  # 56

    nc = bacc.Bacc("TRN2", target_bir_lowering=False, debug=False)
    xp_k = [
        nc.dram_tensor(f"xp{kw}", [B_LOC, C_IN, HP * PITCH], sb_dt, kind="ExternalInput").ap()
        for kw in range(KS)
    ]
    w_d = nc.dram_tensor("w", [C_IN, 9 * C_OUT], w_dt, kind="ExternalInput").ap()
    b_d = nc.dram_tensor("bias", [128, CO_BLOCKS], f32, kind="ExternalInput").ap()
    out_d = nc.dram_tensor("out", [B_LOC, C_OUT, H, W], f32, kind="ExternalOutput").ap()
    out_flat = out_d.rearrange("b c h w -> b c (h w)")

    CHUNKS = [(0, 18, (0, 1)), (16, 18, (2, 3)), (32, 26, (4, 5, 6))]
    t2chunk = {}
    for ci, (r0, nr, ts_) in enumerate(CHUNKS):
        for t in ts_:
            t2chunk[t] = (ci, r0)

    with tile.TileContext(nc) as tc:
        with (
            tc.tile_pool(name="const", bufs=1) as const,
            tc.tile_pool(name="xpool", bufs=18) as xpool,
            tc.tile_pool(name="opool", bufs=4) as opool,
            tc.tile_pool(name="psum", bufs=8, space="PSUM") as psum,
        ):
            import contextlib

            wco = [
                const.tile([C_IN, 9 * 128], w_dt, tag=f"w{co}", name=f"w{co}")
                for co in range(CO_BLOCKS)
            ]
            bt = const.tile([128, CO_BLOCKS], f32)
            for co in range(CO_BLOCKS):
                nc.sync.dma_start(wco[co][:], w_d[:, co * 9 * 128 : (co + 1) * 9 * 128])
            nc.sync.dma_start(bt[:], b_d[:])

            loop_cm = (
                tc.For_i(0, repeats, 1, hint_engines=(mybir.EngineType.PE,))
                if repeats > 1
                else contextlib.nullcontext()
            )
            with loop_cm:
                for b in range(B_LOC):
                    # xc[ci][kw] = chunk ci of the kw-shifted packed copy
                    xc = []
                    for r0, nr, _ts in CHUNKS:
                        row = []
                        for kw in range(KS):
                            xt = xpool.tile([C_IN, 26 * PITCH], sb_dt, tag="xt", name="xt")
                            nc.sync.dma_start(
                                xt[:, : nr * PITCH],
                                xp_k[kw][b, :, r0 * PITCH : (r0 + nr) * PITCH],
                            )
                            row.append(xt)
                        xc.append(row)
                    for t in range(N_TILES):
                        ci, r0 = t2chunk[t]
                        lr = ROWS_PER_TILE * t - r0
                        for co in range(CO_BLOCKS):
                            pt = psum.tile([128, N_TILE], f32, tag="pt", name="pt")
                            for k in range(9):
                                kh, kw = divmod(k, KS)
                                base = (lr + kh) * PITCH
                                nc.tensor.matmul(
                                    pt[:],
                                    wco[co][:, k * 128 : (k + 1) * 128],
                                    xc[ci][kw][:, base : base + N_TILE],
                                    start=(k == 0),
                                    stop=(k == 8),
                                )
                            h0 = ROWS_PER_TILE * t
                            ot = opool.tile([128, N_TILE], f32)
                            nc.vector.tensor_scalar_add(ot[:], pt[:], bt[:, co : co + 1])
                            store_eng = nc.gpsimd if STORE_ENG == "pool" else nc.scalar
                            store_eng.dma_start(
                                out_flat[b, co * 128 : (co + 1) * 128, h0 * W : h0 * W + N_TILE],
                                ot[:],
                            )
    nc.compile()
    return nc


def _build(mode: str, repeats: int = 1) -> bass.Bass:
    if mode.endswith("p"):
        return _build_packed(mode, repeats)
    if mode.endswith("c"):
        return _build_contig(mode, repeats)
    f32 = mybir.dt.float32
    if mode == "bf16":
        sb_dt = mybir.dt.bfloat16
    elif mode in ("f32r", "mixw"):
        sb_dt = mybir.dt.float32r
    else:
        sb_dt = f32
    # "mixw": bf16 weights (half the PE weight-load stream) + f32r activations
    w_dt = mybir.dt.bfloat16 if mode == "mixw" else sb_dt

    nc = bacc.Bacc("TRN2", target_bir_lowering=False, debug=False)
    xp_d = nc.dram_tensor("xp", [B_LOC, C_IN, HP, WP], sb_dt, kind="ExternalInput").ap()
    w_d = nc.dram_tensor("w", [C_IN, 9 * C_OUT], w_dt, kind="ExternalInput").ap()
    b_d = nc.dram_tensor("bias", [128, CO_BLOCKS], f32, kind="ExternalInput").ap()
    out_d = nc.dram_tensor("out", [B_LOC, C_OUT, H, W], f32, kind="ExternalOutput").ap()
    out_flat = out_d.rearrange("b c h w -> b c (h w)")
    xp_rows = xp_d.rearrange("b c h w -> b c (h w)")

    # x chunks per image: A covers output rows 0..15 (padded rows 0..17),
    # B rows 16..31 (padded 16..33), C rows 32..55 (padded 32..57).
    CHUNKS = [(0, 18, (0, 1)), (16, 18, (2, 3)), (32, 26, (4, 5, 6))]
    t2chunk = {}
    for ci, (r0, nr, ts_) in enumerate(CHUNKS):
        for t in ts_:
            t2chunk[t] = (ci, r0)

    with tile.TileContext(nc) as tc:
        with (
            tc.tile_pool(name="const", bufs=1) as const,
            tc.tile_pool(name="xpool", bufs=6) as xpool,
            tc.tile_pool(name="opool", bufs=4) as opool,
            tc.tile_pool(name="psum", bufs=8, space="PSUM") as psum,
        ):
            import contextlib

            loop_cm = (
                tc.For_i(0, repeats, 1, hint_engines=(mybir.EngineType.PE,))
                if repeats > 1
                else contextlib.nullcontext()
            )
            with loop_cm:
                # per-co-block weight tiles: first matmul gates on a 0.6MB DMA.
                # Emission order interleaves image-0's first chunk right after
                # w0 so the PE can start ~2us in; bias is only needed by the
                # first eviction so it loads last.
                wco = [
                    const.tile([C_IN, 9 * 128], w_dt, tag=f"w{co}", name=f"w{co}")
                    for co in range(CO_BLOCKS)
                ]
                bt = const.tile([128, CO_BLOCKS], f32)
                nc.sync.dma_start(wco[0][:], w_d[:, : 9 * 128])
                xc0 = []
                for i, (r0, nr, _ts) in enumerate(CHUNKS):
                    xt = xpool.tile([C_IN, 26, WP], sb_dt, tag="xt", name="xt")
                    nc.sync.dma_start(
                        xt[:, :nr, :], xp_rows[0, :, r0 * WP : (r0 + nr) * WP]
                    )
                    xc0.append(xt)
                    if i == 0:
                        nc.sync.dma_start(wco[1][:], w_d[:, 9 * 128 :])
                nc.sync.dma_start(bt[:], b_d[:])

                for b in range(B_LOC):
                    if b == 0:
                        xc = xc0
                    else:
                        xc = []
                        for r0, nr, _ts in CHUNKS:
                            xt = xpool.tile([C_IN, 26, WP], sb_dt, tag="xt", name="xt")
                            nc.sync.dma_start(
                                xt[:, :nr, :], xp_rows[b, :, r0 * WP : (r0 + nr) * WP]
                            )
                            xc.append(xt)
                    t_groups = (
                        [(0, 1), (2, 3), (4, 5), (6,)]
                        if GROUP2
                        else [(t,) for t in range(N_TILES)]
                    )
                    for tg in t_groups:
                        for co in range(CO_BLOCKS):
                            pts = [
                                psum.tile([128, N_TILE], f32, tag="pt", name="pt")
                                for _ in tg
                            ]
                            for k in range(9):
                                kh, kw = divmod(k, KS)
                                for j, t in enumerate(tg):
                                    h0 = ROWS_PER_TILE * t
                                    ci, r0 = t2chunk[t]
                                    lr = h0 - r0
                                    rhs = xc[ci][:, lr + kh : lr + kh + ROWS_PER_TILE, kw : kw + W]
                                    nc.tensor.matmul(
                                        pts[j][:],
                                        wco[co][:, k * 128 : (k + 1) * 128],
                                        rhs,
                                        start=(k == 0),
                                        stop=(k == 8),
                                    )
                            for j, t in enumerate(tg):
                                h0 = ROWS_PER_TILE * t
                                ot = opool.tile([128, N_TILE], f32)
                                nc.vector.tensor_scalar_add(ot[:], pts[j][:], bt[:, co : co + 1])
                                store_eng = nc.gpsimd if STORE_ENG == "pool" else nc.scalar
                                store_eng.dma_start(
                                    out_flat[b, co * 128 : (co + 1) * 128, h0 * W : h0 * W + N_TILE],
                                    ot[:],
                                )
    nc.compile()
    return nc


def _host_prep(x, kernel, bias, mode: str):
    np_dt = np.float32
    w_np_dt = np.float32
    if mode in ("bf16", "mixw", "bf16c", "bf16p"):
        import ml_dtypes

        w_np_dt = ml_dtypes.bfloat16
        if mode != "mixw":
            np_dt = ml_dtypes.bfloat16

    xp = np.zeros((B, C_IN, HP, WP), dtype=np_dt)
    xp[:, :, 1 : 1 + H, 1 : 1 + W] = x
    if mode.endswith("c"):
        xp = xp.reshape(B, C_IN, HP * WP)
    # w[co, ci, kh, kw] -> w_t[ci, co_blk*9*128 + (kh*3+kw)*128 + co_in]
    w5 = kernel.reshape(CO_BLOCKS, 128, C_IN, KS, KS)
    w_t = np.ascontiguousarray(
        w5.transpose(2, 0, 3, 4, 1).reshape(C_IN, 9 * C_OUT).astype(w_np_dt)
    )
    b_t = np.ascontiguousarray(bias.astype(np.float32).reshape(CO_BLOCKS, 128).T)
    return xp, w_t, b_t


def _make_in_maps(x, kernel, bias, mode: str) -> list[dict]:
    """Per-core input maps for the given mode."""
    xp, w_t, b_t = _host_prep(x, kernel, bias, mode)
    if mode.endswith("p"):
        # three kw-shifted dense copies, row pitch exactly W
        xks = [
            np.ascontiguousarray(xp[:, :, :, kw : kw + W]).reshape(B, C_IN, HP * W)
            for kw in range(KS)
        ]
        return [
            {
                **{f"xp{kw}": xks[kw][c * B_LOC : (c + 1) * B_LOC] for kw in range(KS)},
                "w": w_t,
                "bias": b_t,
            }
            for c in range(N_CORES)
        ]
    return [
        {"xp": xp[c * B_LOC : (c + 1) * B_LOC], "w": w_t, "bias": b_t}
        for c in range(N_CORES)
    ]


def kernel(x, kernel, bias):  # noqa: A002 - names fixed by harness contract
    x = np.asarray(x, dtype=np.float32)
    kernel = np.asarray(kernel, dtype=np.float32)
    bias = np.asarray(bias, dtype=np.float32)

    nc = _build_cached(MODE)
    in_maps = _make_in_maps(x, kernel, bias, MODE)
    res = run_bass_kernel_spmd(nc, in_maps, core_ids=list(range(N_CORES)))
    out = np.concatenate([r["out"] for r in res.results], axis=0)
    return out

